# revision 14
# baseline (speedup 1.0000x reference)
"""DRQN fused kernel for 8 TRN2 NeuronCores.

Data-parallel over batch B=1024 -> 128 rows per core, L=6 timesteps.
Per core the whole net runs locally (no collectives):

    inp = concat(x, pos_onehot)      -- pos is constant (l0=0), folded into
                                        the first-layer biases
    att = sigmoid(relu(inp@aw1+b)@aw2+b)       [768, 1]
    enc = relu(inp@ew1+b)@ew2+b                [768, 1024]
    out = cumsum_L(enc*att)                    [768, 1024]
    act = relu(relu(out@qw1+b)@qw2+b)@qw3+b    [768, 12972]

All matmul operands are bf16 (PSUM accumulation stays fp32); weights are
pre-packed on the host into [128, K, M] chunk layouts so each streaming
load is a single large contiguous-line DMA.  x is transposed on the host
(features on partitions), the qb3 bias is pre-broadcast to 128 partitions,
and the output is stored bf16 [BL, L, ACT] and upcast on the host.
"""

import numpy as np

import concourse.bass as bass
import concourse.mybir as mybir
from concourse import bacc
from concourse.bass_utils import run_bass_kernel_spmd
from concourse.tile import TileContext

F32 = mybir.dt.float32
BF16 = mybir.dt.bfloat16
NPBF16 = mybir.dt.np(BF16)

L, B, N = 6, 1024, 512
G = 6
H, AH, ACT = 1024, 256, 12972
NCORES = 8
BL = B // NCORES          # 128 rows of batch per core
R = L * BL                # 768 rows per core
RG = 2                    # row groups
RGS = R // RG             # 384 rows per group

KN = N // 128   # 4 K-chunks of x features
KH = H // 128   # 8 chunks of hidden features
KA = AH // 128  # 2 chunks of attention features
MROW = R // 128  # 6 row chunks (row chunk m == timestep l)

# qw3 output tiling: 24x512 + 2x342
N_TILES = [512] * 24 + [342, 342]
N_OFFS = np.cumsum([0] + N_TILES)[:-1].tolist()

BUFS = dict(mm=6, att=2, ws=4, q3w=4, st=8, pq=8)
WG = 2  # m-chunks per streamed weight tile for H x H layers
Q3_PREFETCH = 3  # qw3 tiles issued ahead
REPS = 1  # replicate whole computation in one NEFF (for benchmarking)


def build_nc():
    nc = bacc.Bacc()

    xt_h = nc.declare_dram_parameter("xt", [128, KN, R], BF16, isOutput=False)
    aw1_h = nc.declare_dram_parameter("aw1", [128, KN, AH], BF16, isOutput=False)
    aw2_h = nc.declare_dram_parameter("aw2", [128, KA], BF16, isOutput=False)
    ew1_h = nc.declare_dram_parameter("ew1", [128, KN, H], BF16, isOutput=False)
    ew2_h = nc.declare_dram_parameter("ew2", [128, KH, H], BF16, isOutput=False)
    qw1_h = nc.declare_dram_parameter("qw1", [128, KH, H], BF16, isOutput=False)
    qw2_h = nc.declare_dram_parameter("qw2", [128, KH, H], BF16, isOutput=False)
    qw3_h = nc.declare_dram_parameter("qw3", [128, KH, ACT], BF16, isOutput=False)
    qb3_h = nc.declare_dram_parameter("qb3b", [128, ACT], BF16, isOutput=False)
    # biases packed [128, 2+8+8+8+8+1]: ab1 | eb1 | eb2 | qb1 | qb2 | ab2
    NB = KA + 4 * KH + 1
    bp_h = nc.declare_dram_parameter("biasp", [128, NB], F32, isOutput=False)
    ones_h = nc.declare_dram_parameter("ones", [1, 128], BF16, isOutput=False)
    out_h = nc.declare_dram_parameter("out", [BL, L, ACT], BF16, isOutput=True)

    with TileContext(nc) as tc:
      for _rep in range(REPS):
        persist = tc.alloc_tile_pool(name="persist", bufs=1)
        pool_q3 = tc.alloc_tile_pool(name="pool_q3", bufs=1)  # qw3 stream, stores
        ps_a = tc.alloc_tile_pool(name="ps_a", bufs=1, space="PSUM")
        pool_b = tc.alloc_tile_pool(name="pool_b", bufs=1)   # e1t, gt, ew2/qw1 stream
        pool_c = tc.alloc_tile_pool(name="pool_c", bufs=1)   # h1t, qw2 stream
        pool_1 = tc.alloc_tile_pool(name="pool_1", bufs=1)   # xt, aw1, ew1, a1t

        # ---- critical-path loads first on the sync (SP) HWDGE ring ----
        aw1_t = pool_1.tile([128, KN, AH], BF16, name="aw1_t")
        nc.sync.dma_start(out=aw1_t, in_=aw1_h[:])
        xt = pool_1.tile([128, KN, R], BF16, name="xt")
        for k in range(KN):
            nc.sync.dma_start(out=xt[:, k, :], in_=xt_h[:, k, :])
        ew1_t = pool_1.tile([128, KN, H], BF16, name="ew1_t")
        for k in range(KN):
            nc.sync.dma_start(out=ew1_t[:, k, :], in_=ew1_h[:, k, :])

        # ---- constants / biases on the scalar (ACT) HWDGE ring ----
        ones_t = persist.tile([1, 128], BF16, name="ones_t")
        nc.scalar.dma_start(out=ones_t, in_=ones_h[:])
        bp_t = persist.tile([128, NB], F32, name="bp_t")
        nc.scalar.dma_start(out=bp_t, in_=bp_h[:])
        ab1_t = bp_t[:, 0:KA]
        eb1_t = bp_t[:, KA:KA + KH]
        eb2_t = bp_t[:, KA + KH:KA + 2 * KH]
        qb1_t = bp_t[:, KA + 2 * KH:KA + 3 * KH]
        qb2_t = bp_t[:, KA + 3 * KH:KA + 4 * KH]
        ab2_t = bp_t[0:1, KA + 4 * KH:NB]
        aw2_t = pool_1.tile([128, KA], BF16, name="aw2_t")
        nc.scalar.dma_start(out=aw2_t, in_=aw2_h[:])

        att_s = persist.tile([1, R], BF16, name="att_s")
        att_bc = persist.tile([128, R], BF16, name="att_bc")
        h2t = [persist.tile([128, R], BF16, name=f"h2t{k}", tag="h2t", bufs=KH)
               for k in range(KH)]
        qb3_t = persist.tile([128, ACT], BF16, name="qb3_t")

        # ---- attention branch ----
        a1t = [pool_1.tile([128, R], BF16, name=f"a1t{m}", tag="a1t", bufs=KA)
               for m in range(KA)]
        for m in range(KA):
            pls = [ps_a.tile([128, RGS], F32, name="a1ps", tag="mm", bufs=BUFS["mm"])
                   for _ in range(RG)]
            for k in range(KN):
                for g in range(RG):
                    nc.tensor.matmul(
                        pls[g], aw1_t[:, k, m * 128:(m + 1) * 128],
                        xt[:, k, g * RGS:(g + 1) * RGS],
                        start=(k == 0), stop=(k == KN - 1))
            for g in range(RG):
                nc.scalar.activation(
                    a1t[m][:, g * RGS:(g + 1) * RGS], pls[g],
                    mybir.ActivationFunctionType.Relu, bias=ab1_t[:, m:m + 1])

        for g in range(RG):
            aps = ps_a.tile([1, RGS], F32, name="aps", tag="att", bufs=BUFS["att"])
            for k in range(KA):
                nc.tensor.matmul(
                    aps, aw2_t[:, k:k + 1], a1t[k][:, g * RGS:(g + 1) * RGS],
                    start=(k == 0), stop=(k == KA - 1))
            nc.scalar.activation(
                att_s[:, g * RGS:(g + 1) * RGS], aps,
                mybir.ActivationFunctionType.Sigmoid, bias=ab2_t[:, 0:1])
            bps = ps_a.tile([128, RGS], F32, name="bps", tag="att", bufs=BUFS["att"])
            nc.tensor.matmul(
                bps, ones_t, att_s[:, g * RGS:(g + 1) * RGS],
                start=True, stop=True)
            nc.vector.tensor_copy(att_bc[:, g * RGS:(g + 1) * RGS], bps)

        # qb3 broadcast: loaded once, consumed in the q3 tail
        nc.scalar.dma_start(out=qb3_t, in_=qb3_h[:])

        # ---- encoder layer 1 ----
        e1t = [pool_b.tile([128, R], BF16, name=f"e1t{m}", tag="e1t", bufs=KH)
               for m in range(KH)]
        for m in range(KH):
            pls = [ps_a.tile([128, RGS], F32, name="e1ps", tag="mm", bufs=BUFS["mm"])
                   for _ in range(RG)]
            for k in range(KN):
                for g in range(RG):
                    nc.tensor.matmul(
                        pls[g], ew1_t[:, k, m * 128:(m + 1) * 128],
                        xt[:, k, g * RGS:(g + 1) * RGS],
                        start=(k == 0), stop=(k == KN - 1))
            for g in range(RG):
                nc.scalar.activation(
                    e1t[m][:, g * RGS:(g + 1) * RGS], pls[g],
                    mybir.ActivationFunctionType.Relu, bias=eb1_t[:, m:m + 1])

        pool_1.release()

        # ---- qw3 prefetch machinery ----
        q3wts = {}

        def load_q3(nt):
            n0, nn = N_OFFS[nt], N_TILES[nt]
            w = pool_q3.tile([128, KH, 512], BF16, name="qw3s", tag="qw3s",
                             bufs=BUFS["q3w"])
            nc.sync.dma_start(out=w[:, :, :nn], in_=qw3_h[:, :, n0:n0 + nn])
            q3wts[nt] = w

        # ---- encoder layer 2 + gate + cumsum over L ----
        gt = [pool_b.tile([128, R], BF16, name=f"gt{m}", tag="gt", bufs=KH)
              for m in range(KH)]
        for mg in range(KH // WG):
            wts = pool_b.tile([128, KH, WG * 128], BF16, name="ew2s", tag="ws",
                              bufs=BUFS["ws"])
            nc.sync.dma_start(
                out=wts, in_=ew2_h[:, :, mg * WG * 128:(mg + 1) * WG * 128])
            for mi in range(WG):
                m = mg * WG + mi
                pls = [ps_a.tile([128, RGS], F32, name="e2ps", tag="mm", bufs=BUFS["mm"])
                       for _ in range(RG)]
                for k in range(KH):
                    for g in range(RG):
                        nc.tensor.matmul(
                            pls[g], wts[:, k, mi * 128:(mi + 1) * 128],
                            e1t[k][:, g * RGS:(g + 1) * RGS],
                            start=(k == 0), stop=(k == KH - 1))
                for g in range(RG):
                    # gt = (psum + eb2) * att
                    nc.vector.scalar_tensor_tensor(
                        gt[m][:, g * RGS:(g + 1) * RGS], pls[g], eb2_t[:, m:m + 1],
                        att_bc[:, g * RGS:(g + 1) * RGS],
                        op0=mybir.AluOpType.add, op1=mybir.AluOpType.mult)
                # running sum over the 6 timesteps (128-row blocks of free dim)
                for l in range(1, L):
                    nc.vector.tensor_add(
                        gt[m][:, l * 128:(l + 1) * 128],
                        gt[m][:, l * 128:(l + 1) * 128],
                        gt[m][:, (l - 1) * 128:l * 128])

        # ---- q head layer 1 ----
        h1t = [pool_c.tile([128, R], BF16, name=f"h1t{m}", tag="h1t", bufs=KH)
               for m in range(KH)]
        for mg in range(KH // WG):
            wts = pool_b.tile([128, KH, WG * 128], BF16, name="qw1s", tag="ws",
                              bufs=BUFS["ws"])
            nc.sync.dma_start(
                out=wts, in_=qw1_h[:, :, mg * WG * 128:(mg + 1) * WG * 128])
            for mi in range(WG):
                m = mg * WG + mi
                pls = [ps_a.tile([128, RGS], F32, name="q1ps", tag="mm", bufs=BUFS["mm"])
                       for _ in range(RG)]
                for k in range(KH):
                    for g in range(RG):
                        nc.tensor.matmul(
                            pls[g], wts[:, k, mi * 128:(mi + 1) * 128],
                            gt[k][:, g * RGS:(g + 1) * RGS],
                            start=(k == 0), stop=(k == KH - 1))
                for g in range(RG):
                    nc.scalar.activation(
                        h1t[m][:, g * RGS:(g + 1) * RGS], pls[g],
                        mybir.ActivationFunctionType.Relu, bias=qb1_t[:, m:m + 1])

        # prefetch the first qw3 tiles while q2 runs
        for nt in range(Q3_PREFETCH):
            load_q3(nt)

        # ---- q head layer 2 ----
        for mg in range(KH // WG):
            wts = pool_c.tile([128, KH, WG * 128], BF16, name="qw2s", tag="ws",
                              bufs=BUFS["ws"])
            nc.sync.dma_start(
                out=wts, in_=qw2_h[:, :, mg * WG * 128:(mg + 1) * WG * 128])
            for mi in range(WG):
                m = mg * WG + mi
                pls = [ps_a.tile([128, RGS], F32, name="q2ps", tag="mm", bufs=BUFS["mm"])
                       for _ in range(RG)]
                for k in range(KH):
                    for g in range(RG):
                        nc.tensor.matmul(
                            pls[g], wts[:, k, mi * 128:(mi + 1) * 128],
                            h1t[k][:, g * RGS:(g + 1) * RGS],
                            start=(k == 0), stop=(k == KH - 1))
                for g in range(RG):
                    nc.scalar.activation(
                        h2t[m][:, g * RGS:(g + 1) * RGS], pls[g],
                        mybir.ActivationFunctionType.Relu, bias=qb2_t[:, m:m + 1])

        pool_c.release()
        pool_b.release()
        ps_a.release()

        # ---- action layer: out[b, l, :] = h2 @ qw3 + qb3 ----
        ps_b = tc.alloc_tile_pool(name="ps_b", bufs=1, space="PSUM")

        for nt, (n0, nn) in enumerate(zip(N_OFFS, N_TILES)):
            wts = q3wts.pop(nt)
            if nt + Q3_PREFETCH < len(N_TILES):
                load_q3(nt + Q3_PREFETCH)
            for m in range(MROW):
                pq = ps_b.tile([128, 512], F32, name="pq", tag="pq", bufs=BUFS["pq"])
                for k in range(KH):
                    nc.tensor.matmul(
                        pq[:, :nn], h2t[k][:, m * 128:(m + 1) * 128],
                        wts[:, k, :nn], start=(k == 0), stop=(k == KH - 1))
                st = pool_q3.tile([128, 512], BF16, name="st", tag="st",
                                  bufs=BUFS["st"])
                nc.vector.tensor_add(st[:, :nn], pq[:, :nn],
                                     qb3_t[:, n0:n0 + nn])
                nc.scalar.dma_start(out=out_h[:, m, n0:n0 + nn], in_=st[:, :nn])

        pool_q3.release()
        ps_b.release()
        persist.release()

    nc.finalize()
    return nc


_NC_CACHE = {}
_BENCH_CACHE = {}
_PREP_CACHE = {}


def _get_nc(reps=1):
    global REPS
    if reps not in _NC_CACHE:
        old = REPS
        REPS = reps
        try:
            _NC_CACHE[reps] = build_nc()
        finally:
            REPS = old
    return _NC_CACHE[reps]


def _pack_w(w, kchunks):
    """[K, M] fp32 -> [128, kchunks, M] bf16 (k-chunk of rows on free dim)."""
    K, M = w.shape
    assert K == kchunks * 128
    return np.ascontiguousarray(
        w.reshape(kchunks, 128, M).transpose(1, 0, 2)).astype(NPBF16)


def _prep_in_maps(inputs):
    f = lambda a: np.asarray(a, dtype=np.float32)
    x = f(inputs["x"])
    aw1 = f(inputs["aw1"])
    ab1e = f(inputs["ab1"]) + aw1[N]          # fold pos one-hot (l0=0)
    ew1 = f(inputs["ew1"])
    eb1e = f(inputs["eb1"]) + ew1[N]
    qb3b = np.broadcast_to(f(inputs["qb3"]), (128, ACT))
    # biases packed [128, 2+8+8+8+8+1]: ab1 | eb1 | eb2 | qb1 | qb2 | ab2
    bp = np.concatenate([
        ab1e.reshape(KA, 128).T,
        eb1e.reshape(KH, 128).T,
        f(inputs["eb2"]).reshape(KH, 128).T,
        f(inputs["qb1"]).reshape(KH, 128).T,
        f(inputs["qb2"]).reshape(KH, 128).T,
        np.broadcast_to(f(inputs["ab2"]), (128, 1)),
    ], axis=1)
    shared = {
        "aw1": _pack_w(aw1[:N], KN),
        "aw2": _pack_w(f(inputs["aw2"]), KA).reshape(128, KA),
        "ew1": _pack_w(ew1[:N], KN),
        "ew2": _pack_w(f(inputs["ew2"]), KH),
        "qw1": _pack_w(f(inputs["qw1"]), KH),
        "qw2": _pack_w(f(inputs["qw2"]), KH),
        "qw3": _pack_w(f(inputs["qw3"]), KH),
        "qb3b": np.ascontiguousarray(qb3b.astype(NPBF16)),
        "biasp": np.ascontiguousarray(bp),
        "ones": np.ones((1, 128), dtype=NPBF16),
    }
    in_maps = []
    for c in range(NCORES):
        m = dict(shared)
        # x shard [L, BL, N] -> transposed [N, L*BL] -> packed [128, KN, R]
        xs = x[:, c * BL:(c + 1) * BL, :].transpose(2, 0, 1).reshape(N, R)
        m["xt"] = _pack_w(xs, KN)
        in_maps.append(m)
    return in_maps


def _bf16_to_f32(a):
    return (a.view(np.uint16).astype(np.uint32) << 16).view(np.float32)


def run(inputs, **kwargs):
    import os
    os.environ.setdefault("BASS_NEVER_TRACE", "1")
    nc = _get_nc()
    in_maps = _prep_in_maps(inputs)
    res = run_bass_kernel_spmd(nc, in_maps, list(range(NCORES)), **kwargs)
    out = np.empty((L, B, ACT), dtype=np.float32)
    for c in range(NCORES):
        # device out is [BL, L, ACT] bf16
        o = _bf16_to_f32(np.asarray(res.results[c]["out"]))
        out[:, c * BL:(c + 1) * BL, :] = o.transpose(1, 0, 2)
    return out, res


def kernel(**inputs) -> np.ndarray:
    out, _ = run(inputs)
    return out


def bench(inputs, iters=20, warmup=3, reps=1):
    """Steady-state per-call wall time of the compiled 8-core NEFF with
    device-resident inputs (pipelined dispatch, single block at end)."""
    import time

    if reps in _BENCH_CACHE:
        sharded, concat_in, mk_zeros = _BENCH_CACHE[reps]
        import jax
        # chain-donate: each call consumes the previous call's output buffers,
        # so the timed loop does zero allocation.
        cur = mk_zeros()
        for _ in range(max(warmup, 1)):
            cur = sharded(*concat_in, *cur)
        jax.block_until_ready(cur)
        t0 = time.perf_counter()
        for _ in range(iters):
            cur = sharded(*concat_in, *cur)
        jax.block_until_ready(cur)
        t1 = time.perf_counter()
        return (t1 - t0) / iters * 1e9

    import jax
    from jax.sharding import Mesh, NamedSharding, PartitionSpec
    from jax.experimental.shard_map import shard_map

    import concourse.mybir as mybir_
    from concourse import bass2jax

    bass2jax.install_neuronx_cc_hook()
    nc = _get_nc(reps)
    in_maps = _prep_in_maps(inputs)

    partition_name = nc.partition_id_tensor.name if nc.partition_id_tensor else None
    in_names, out_names, out_avals, zero_shapes = [], [], [], []
    for alloc in nc.m.functions[0].allocations:
        if not isinstance(alloc, mybir_.MemoryLocationSet):
            continue
        name = alloc.memorylocations[0].name
        if alloc.kind == "ExternalInput":
            if name != partition_name:
                in_names.append(name)
        elif alloc.kind == "ExternalOutput":
            out_names.append(name)
            shape = tuple(alloc.tensor_shape)
            dtype = mybir_.dt.np(alloc.dtype)
            out_avals.append(jax.core.ShapedArray(shape, dtype))
            zero_shapes.append((shape, dtype))
    n_params = len(in_names)
    n_outs = len(out_avals)
    all_names = list(in_names) + list(out_names)
    if partition_name is not None:
        all_names.append(partition_name)
    donate = tuple(range(n_params, n_params + n_outs))

    def _body(*args):
        operands = list(args)
        if partition_name is not None:
            operands.append(bass2jax.partition_id_tensor())
        return tuple(bass2jax._bass_exec_p.bind(
            *operands,
            out_avals=tuple(out_avals),
            in_names=tuple(all_names),
            out_names=tuple(out_names),
            lowering_input_output_aliases=(),
            sim_require_finite=True,
            sim_require_nnan=True,
            nc=nc,
        ))

    devices = jax.devices()[:NCORES]
    mesh = Mesh(np.asarray(devices), ("core",))
    spec = NamedSharding(mesh, PartitionSpec("core"))
    in_specs = (PartitionSpec("core"),) * (n_params + n_outs)
    out_specs = (PartitionSpec("core"),) * n_outs
    sharded = jax.jit(
        shard_map(_body, mesh=mesh, in_specs=in_specs, out_specs=out_specs,
                  check_rep=False),
        donate_argnums=donate, keep_unused=True)

    concat_in = [
        jax.device_put(
            np.concatenate([np.asarray(in_maps[c][n]) for c in range(NCORES)],
                           axis=0), spec)
        for n in in_names
    ]

    def mk_zeros():
        import jax.numpy as jnp
        return [jax.device_put(
                    jnp.zeros((NCORES * s[0], *s[1:]), dt), spec)
                for (s, dt) in zero_shapes]

    _BENCH_CACHE[reps] = (sharded, concat_in, mk_zeros)
    return bench(inputs, iters=iters, warmup=warmup, reps=reps)
